# revision 13
# baseline (speedup 1.0000x reference)
"""Trainium2 Bass kernel for a dense transformer encoder layer.

Math note: in this layer, k is replaced by mean_s(q) before the attention
matmul, so every attention logit row is constant -> softmax is exactly
uniform -> attention output equals the mean of v over the sequence,
broadcast to every position.  The entire attention block collapses to a
per-batch vector:

    a[b] = M[b] @ Wcomb + bcomb,   M[b] = sum_t (x_t - mu_t) * rstd_t
         = m1 @ Wcomb - c * colsum(Wcomb) + bcomb
    with m1 = sum_t rstd_t * x_t   (8 N=512 matmuls, lhsT = rstd column)
         c  = sum_t rstd_t * mu_t  (8 N=1 matmuls)

LN2 never materializes: mm1 runs on  xT2 = (x^T + a) * rstd2_row  (one DVE
scalar_tensor_tensor per e-chunk), so  W1 @ xT2  is already rstd2-scaled;
the remaining rank-1 term  -s_t * r_f  (s = mu2*rstd2, r = colsum(w1_eff))
is fused into the PSUM by one DVE pass, and b1 rides the Gelu bias.

x^T arrives pre-transposed from the host (bf16), killing all PE transposes.
Own-half stats run on a bf16 copy (xob) so the f32 x (residual only) can
stream in late at low DMA priority.  rstd2's partition-broadcast tile is
built exactly via a K=2 bf16 matmul on a hi/lo split.  mm2 is interleaved
into the mm1 f-chunk loop (skewed by one chunk) so the PE never waits on
the Gelu chain and output tiles complete right after the last chunk.

DMA plan: ~11 coalesced transfers spread over sync / scalar / gpsimd
queues; x (bf16) first at full bandwidth, w1/w2 next, f32 x last.
"""

import numpy as np
import ml_dtypes

import concourse.bass as bass
import concourse.mybir as mybir
from concourse import bacc
from concourse.tile import TileContext
from concourse.bass_utils import run_bass_kernel_spmd
from concourse.masks import make_identity

B, S, E = 4, 1024, 512
FF = 4 * E
EPS = 1e-5
P = 128
NCORES = 8
EC = E // P      # 4  e-chunks of 128
FC = FF // P     # 16 f-chunks of 128
OWN = 4          # own token tiles of 128
HS = S // 2      # 512 own tokens

WARM = 8         # PE warmup matmuls before real work

F32 = mybir.dt.float32
BF16 = mybir.dt.bfloat16
BF = ml_dtypes.bfloat16
AF = mybir.ActivationFunctionType
OP = mybir.AluOpType


def _build():
    nc = bacc.Bacc("TRN2", target_bir_lowering=False, debug=False,
                   num_devices=NCORES)

    # all inputs pre-shaped on host to the exact SBUF layout
    xob = nc.dram_tensor("xob", [P, OWN, E], BF16, kind="ExternalInput")
    xhb = nc.dram_tensor("xhb", [P, OWN, E], BF16, kind="ExternalInput")
    xot = nc.dram_tensor("xot", [P, EC, HS], BF16, kind="ExternalInput")
    xo32 = nc.dram_tensor("xo32", [P, OWN, E], F32, kind="ExternalInput")
    cw = nc.dram_tensor("cw", [P, EC, E], BF16, kind="ExternalInput")
    w1 = nc.dram_tensor("w1", [P, EC, FC, P], BF16, kind="ExternalInput")
    w2 = nc.dram_tensor("w2", [P, FC, E], BF16, kind="ExternalInput")
    misc = nc.dram_tensor("misc", [P, 2 * FC], F32, kind="ExternalInput")
    rows32 = nc.dram_tensor("rows32", [1, 2 * E], F32, kind="ExternalInput")
    b2r = nc.dram_tensor("b2r", [1, E], BF16, kind="ExternalInput")
    out = nc.dram_tensor("out", [P, OWN, E], F32, kind="ExternalOutput")

    with TileContext(nc) as tc:
        with (
            tc.tile_pool(name="pers", bufs=1) as pers,
            tc.tile_pool(name="stats", bufs=8) as stats,
            tc.tile_pool(name="psS", bufs=2, space="PSUM") as psS,
            tc.tile_pool(name="psM", bufs=2, space="PSUM") as psM,
            tc.tile_pool(name="psO", bufs=4, space="PSUM") as psO,
        ):
            # ---- constants (no DMA deps) ----
            eps_t = pers.tile([P, 1], F32, tag="eps")
            nc.vector.memset(eps_t, EPS)
            ones1 = pers.tile([1, P], BF16, tag="ones1")
            nc.vector.memset(ones1, 1.0)
            ones2 = pers.tile([2, P], BF16, tag="ones2")
            nc.vector.memset(ones2, 1.0)
            onecol = pers.tile([1, 1], BF16, tag="onecol")
            nc.vector.memset(onecol, 1.0)
            onecol128 = pers.tile([P, 1], BF16, tag="onecol128")
            nc.vector.memset(onecol128, 1.0)
            ones2col = pers.tile([2, 1], BF16, tag="ones2col")
            nc.vector.memset(ones2col, 1.0)
            junk = pers.tile([P, E], BF16, tag="junk")
            nc.vector.memset(junk, 0.0)
            id_b = pers.tile([P, P], BF16, tag="id_b")
            make_identity(nc, id_b)

            # ---- DMA issues, per engine stream ----
            # sync queue: xob (own bf16), w1, (output tiles at the end)
            xob_sb = pers.tile([P, OWN, E], BF16, tag="xob")
            nc.sync.dma_start(out=xob_sb[:, 0:2, :], in_=xob[:, 0:2, :])
            nc.sync.dma_start(out=xob_sb[:, 2:4, :], in_=xob[:, 2:4, :])
            w1_sb = pers.tile([P, EC, FC, P], BF16, tag="w1")
            nc.sync.dma_start(out=w1_sb[:], in_=w1[:])

            # scalar queue: xhb, cw, w2; ACT tables triggered between issues
            xhb_sb = pers.tile([P, OWN, E], BF16, tag="xhb")
            nc.scalar.dma_start(out=xhb_sb[:, 0:2, :], in_=xhb[:, 0:2, :])
            actw = pers.tile([P, 1], F32, tag="actw")
            nc.scalar.activation(out=actw[:], in_=eps_t[:], func=AF.Sqrt,
                                 bias=eps_t[:], scale=1.0)
            nc.scalar.dma_start(out=xhb_sb[:, 2:4, :], in_=xhb[:, 2:4, :])
            cw_sb = pers.tile([P, EC, E], BF16, tag="cw")
            nc.scalar.dma_start(out=cw_sb[:], in_=cw[:])
            nc.scalar.activation(out=actw[:], in_=eps_t[:], func=AF.Gelu,
                                 bias=eps_t[:], scale=1.0)
            w2_sb = pers.tile([P, FC, E], BF16, tag="w2")
            nc.scalar.dma_start(out=w2_sb[:], in_=w2[:])

            # gpsimd queue (SWDGE): xot, small consts, then low-prio f32 x
            xot_sb = pers.tile([P, EC, HS], BF16, tag="xot")
            nc.gpsimd.dma_start(out=xot_sb[:], in_=xot[:])
            rows_sb = pers.tile([1, 2 * E], F32, tag="rows32")
            nc.gpsimd.dma_start(out=rows_sb[:], in_=rows32[:])
            misc_sb = pers.tile([P, 2 * FC], F32, tag="misc")
            nc.gpsimd.dma_start(out=misc_sb[:], in_=misc[:])
            b2_sb = pers.tile([1, E], BF16, tag="b2r")
            nc.gpsimd.dma_start(out=b2_sb[:], in_=b2r[:])
            xo32_sb = pers.tile([P, OWN, E], F32, tag="xo32")
            nc.gpsimd.dma_start(out=xo32_sb[:], in_=xo32[:])

            # ---- PE warmup while x lands ----
            for wi in range(WARM):
                pW = psM.tile([P, E], F32, tag="pM", name=f"pW{wi}")
                nc.tensor.matmul(pW[:], lhsT=junk[:, 0:P], rhs=junk[:],
                                 start=True, stop=True)

            # ---- stage A: LN1 stats -> m1 row + c scalar over 1024 toks ----
            m1p = psS.tile([1, E], F32, tag="psS", name="m1p")
            acc_c = pers.tile([P, 1], F32, tag="acc_c")
            tiles = [(xob_sb, 0), (xob_sb, 1), (xhb_sb, 0), (xhb_sb, 1),
                     (xob_sb, 2), (xob_sb, 3), (xhb_sb, 2), (xhb_sb, 3)]
            for n, (src, i) in enumerate(tiles):
                st = stats.tile([P, 6], F32, tag="st")
                nc.vector.bn_stats(out=st[:], in_=src[:, i, :])
                mv = stats.tile([P, 2], F32, tag="mv")
                nc.vector.bn_aggr(out=mv[:], in_=st[:])
                srt = stats.tile([P, 1], F32, tag="srt")
                nc.scalar.activation(out=srt[:], in_=mv[:, 1:2],
                                     func=AF.Sqrt, bias=eps_t[:], scale=1.0)
                rstd = stats.tile([P, 1], F32, tag="rstd")
                nc.vector.reciprocal(out=rstd[:], in_=srt[:])
                pr = stats.tile([P, 2], BF16, tag="pr", bufs=4)
                nc.vector.tensor_copy(pr[:, 0:1], rstd[:])
                cpr = stats.tile([P, 1], F32, tag="cpr")
                nc.vector.tensor_mul(cpr[:], mv[:, 0:1], rstd[:])
                if n == 0:
                    nc.vector.tensor_copy(acc_c[:], cpr[:])
                else:
                    nc.vector.tensor_add(acc_c[:], acc_c[:], cpr[:])
                nc.tensor.matmul(m1p[:], lhsT=pr[:, 0:1], rhs=src[:, i, :],
                                 start=(n == 0), stop=(n == 7))

            # ---- stage B: a2 = m1 @ Wcomb + c*ncsum + bcomb ----
            m1row = stats.tile([1, E], BF16, tag="m1row")
            nc.vector.tensor_copy(m1row[:], m1p[:])
            accb = stats.tile([P, 1], BF16, tag="accb")
            nc.vector.tensor_copy(accb[:], acc_c[:])
            cp = psS.tile([1, 1], F32, tag="psS", name="cp")
            nc.tensor.matmul(cp[:], lhsT=accb[:], rhs=onecol128[:],
                             start=True, stop=True)
            c_sb = stats.tile([1, 1], F32, tag="c_sb")
            nc.vector.tensor_copy(c_sb[:], cp[:])
            m1c = stats.tile([P, EC], BF16, tag="m1c")
            for k in range(EC):
                pc = psS.tile([P, 1], F32, tag="psS", name=f"m1c{k}")
                nc.tensor.matmul(pc[:], lhsT=m1row[:, k * P:(k + 1) * P],
                                 rhs=onecol[:], start=True, stop=True)
                nc.vector.tensor_copy(m1c[:, k:k + 1], pc[:])
            pA = psS.tile([1, E], F32, tag="psS", name="pA")
            for k in range(EC):
                nc.tensor.matmul(pA[:], lhsT=m1c[:, k:k + 1],
                                 rhs=cw_sb[:, k, :],
                                 start=(k == 0), stop=(k == EC - 1))
            # a2 = (ncsum * c) + pA, then += bcomb  (rows_sb: 0=bcomb 1=ncsum)
            a2 = stats.tile([1, E], F32, tag="a2")
            nc.vector.scalar_tensor_tensor(out=a2[:], in0=rows_sb[:, E:2 * E],
                                           scalar=c_sb[:], in1=pA[:],
                                           op0=OP.mult, op1=OP.add)
            nc.vector.tensor_add(a2[:], a2[:], rows_sb[:, 0:E])
            # hi/lo split of a2 for exact bf16-matmul broadcast
            a2hi = stats.tile([1, E], BF16, tag="a2hi")
            nc.vector.tensor_copy(a2hi[:], a2[:])
            a2lo = stats.tile([1, E], F32, tag="a2lo")
            nc.vector.tensor_sub(a2lo[:], a2[:], a2hi[:])
            a2lob = stats.tile([1, E], BF16, tag="a2lob")
            nc.vector.tensor_copy(a2lob[:], a2lo[:])
            # a as per-e-chunk columns (for the xT2 add)
            acol = stats.tile([P, EC], F32, tag="acol")
            for k in range(EC):
                pc = psS.tile([P, 1], F32, tag="psS", name=f"acol{k}")
                nc.tensor.matmul(pc[:], lhsT=a2hi[:, k * P:(k + 1) * P],
                                 rhs=onecol[:], start=True, stop=False)
                nc.tensor.matmul(pc[:], lhsT=a2lob[:, k * P:(k + 1) * P],
                                 rhs=onecol[:], start=False, stop=True)
                nc.vector.tensor_copy(acol[:, k:k + 1], pc[:])
            # pBC = broadcast(a2) to 128 rows; keep an SBUF f32 copy
            pBC = psS.tile([P, E], F32, tag="psS", name="pBC")
            nc.tensor.matmul(pBC[:], lhsT=ones1[:], rhs=a2hi[:],
                             start=True, stop=False)
            nc.tensor.matmul(pBC[:], lhsT=ones1[:], rhs=a2lob[:],
                             start=False, stop=True)
            pBC_sb = pers.tile([P, E], F32, tag="pBC_sb")
            nc.vector.tensor_copy(pBC_sb[:], pBC[:])

            # ---- stage C: LN2 stats on bf16 x2; build S/R broadcast rows ---
            x2b = pers.tile([P, OWN, E], BF16, tag="x2b")
            rows3 = pers.tile([2, HS], BF16, tag="rows3")
            rows1s = pers.tile([1, HS], BF16, tag="rows1s")
            for i in range(OWN):
                nc.vector.tensor_add(x2b[:, i, :], xob_sb[:, i, :], pBC[:])
            for i in range(OWN):
                st = stats.tile([P, 6], F32, tag="st")
                nc.vector.bn_stats(out=st[:], in_=x2b[:, i, :])
                mv = stats.tile([P, 2], F32, tag="mv")
                nc.vector.bn_aggr(out=mv[:], in_=st[:])
                srt = stats.tile([P, 1], F32, tag="srt")
                nc.scalar.activation(out=srt[:], in_=mv[:, 1:2],
                                     func=AF.Sqrt, bias=eps_t[:], scale=1.0)
                r2 = stats.tile([P, 1], F32, tag="rstd")
                nc.vector.reciprocal(out=r2[:], in_=srt[:])
                pk = stats.tile([P, 2], BF16, tag="pk", bufs=4)
                nc.vector.tensor_copy(pk[:, 0:1], r2[:])           # hi
                lo = stats.tile([P, 1], F32, tag="lo")
                nc.vector.tensor_sub(lo[:], r2[:], pk[:, 0:1])
                nc.vector.tensor_copy(pk[:, 1:2], lo[:])           # lo
                sk = stats.tile([P, 1], BF16, tag="sk", bufs=4)
                s_f = stats.tile([P, 1], F32, tag="s_f")
                nc.vector.tensor_mul(s_f[:], mv[:, 0:1], r2[:])
                nc.vector.tensor_copy(sk[:], s_f[:])               # s
                pT = psS.tile([2, P], BF16, tag="psS", name=f"pT{i}")
                nc.tensor.transpose(pT[:], in_=pk[:], identity=id_b[:])
                nc.vector.tensor_copy(rows3[:, i * P:(i + 1) * P], pT[:])
                pTs = psS.tile([1, P], BF16, tag="psS", name=f"pTs{i}")
                nc.tensor.transpose(pTs[:], in_=sk[:], identity=id_b[:])
                nc.vector.tensor_copy(rows1s[:, i * P:(i + 1) * P], pTs[:])

            # S_b / R_b broadcast tiles
            pR = psS.tile([P, HS], F32, tag="psS", name="pR")
            nc.tensor.matmul(pR[:], lhsT=ones2[:], rhs=rows3[:],
                             start=True, stop=True)
            pS = psS.tile([P, HS], F32, tag="psS", name="pS")
            nc.tensor.matmul(pS[:], lhsT=ones1[:], rhs=rows1s[:],
                             start=True, stop=True)
            S_sb = pers.tile([P, HS], F32, tag="S_sb")
            nc.vector.tensor_copy(S_sb[:], pS[:])

            # xT2 = (xot + a_col) * rstd2_row
            xT2 = pers.tile([P, EC, HS], BF16, tag="xT2")
            for k in range(EC):
                nc.vector.scalar_tensor_tensor(out=xT2[:, k, :],
                                               in0=xot_sb[:, k, :],
                                               scalar=acol[:, k:k + 1],
                                               in1=pR[:],
                                               op0=OP.add, op1=OP.mult)

            # ---- MLP: mm1 + corrections + gelu, mm2 skewed by one chunk ---
            h1 = pers.tile([P, FC, HS], BF16, tag="h1")
            pO = [psO.tile([P, E], F32, tag="psO", name=f"pO{i}")
                  for i in range(OWN)]

            def emit_mm2(g):
                for i in range(OWN):
                    nc.tensor.matmul(pO[i][:],
                                     lhsT=h1[:, g, i * P:(i + 1) * P],
                                     rhs=w2_sb[:, g, :],
                                     start=(g == 0), stop=False)

            for f in range(FC):
                pM = psM.tile([P, HS], F32, tag="pM")
                for k in range(EC):
                    nc.tensor.matmul(pM[:], lhsT=w1_sb[:, k, f, :],
                                     rhs=xT2[:, k, :],
                                     start=(k == 0), stop=(k == EC - 1))
                nc.vector.scalar_tensor_tensor(out=pM[:], in0=S_sb[:],
                                               scalar=misc_sb[:, f:f + 1],
                                               in1=pM[:],
                                               op0=OP.mult, op1=OP.add)
                nc.scalar.activation(out=h1[:, f, :], in_=pM[:], func=AF.Gelu,
                                     bias=misc_sb[:, FC + f:FC + f + 1],
                                     scale=1.0)
                if f > 0:
                    emit_mm2(f - 1)
            emit_mm2(FC - 1)

            # x2 f32 (gpsimd, off critical path) + bias + residual + out DMA
            x2f = pers.tile([P, OWN, E], F32, tag="x2f")
            for i in range(OWN):
                nc.gpsimd.tensor_add(x2f[:, i, :], xo32_sb[:, i, :],
                                     pBC_sb[:])
            for i in range(OWN):
                nc.tensor.matmul(pO[i][:], lhsT=ones1[:], rhs=b2_sb[:],
                                 start=False, stop=True)
                nc.vector.tensor_add(x2f[:, i, :], x2f[:, i, :], pO[i][:])
                nc.sync.dma_start(out=out[:, i, :], in_=x2f[:, i, :])

    nc.compile()
    return nc


_CACHE = {}
LAST_RESULT = None


def _program():
    if "nc" not in _CACHE:
        _CACHE["nc"] = _build()
    return _CACHE["nc"]


def kernel(x, ln1_w, ln1_b, qkv_w, qkv_b, out_w, out_b,
           ln2_w, ln2_b, fc1_w, fc1_b, fc2_w, fc2_b, **extra):
    import os
    global LAST_RESULT

    f32 = np.float32
    x = np.asarray(x, f32)
    qkv_w = np.asarray(qkv_w, np.float64)
    qkv_b = np.asarray(qkv_b, np.float64)
    out_w = np.asarray(out_w, np.float64)
    out_b = np.asarray(out_b, np.float64)
    ln1_w = np.asarray(ln1_w, np.float64)
    ln1_b = np.asarray(ln1_b, np.float64)
    ln2_w = np.asarray(ln2_w, np.float64)
    ln2_b = np.asarray(ln2_b, np.float64)
    fc1_w = np.asarray(fc1_w, np.float64)
    fc1_b = np.asarray(fc1_b, np.float64)
    fc2_w = np.asarray(fc2_w, f32)
    fc2_b = np.asarray(fc2_b, f32)

    # attention collapse folds (see module docstring)
    WvT = qkv_w[2 * E:3 * E].T
    wv_eff = (ln1_w[:, None] / S) * WvT
    bv_eff = ln1_b @ WvT + qkv_b[2 * E:3 * E]
    WoT = out_w.T
    Wcomb = wv_eff @ WoT
    bcomb = bv_eff @ WoT + out_b
    ncsum = -Wcomb.sum(axis=0)
    # LN2 affine folded into fc1; r = colsums for the rank-1 correction
    w1_eff = ln2_w[:, None] * fc1_w.T              # [E, FF]
    b1_eff = fc1_b + ln2_b @ fc1_w.T
    r_eff = w1_eff.sum(axis=0)

    cw_h = np.ascontiguousarray(
        Wcomb.reshape(EC, P, E).transpose(1, 0, 2)).astype(BF)
    w1_h = np.ascontiguousarray(
        w1_eff.reshape(EC, P, FC, P).transpose(1, 0, 2, 3)).astype(BF)
    w2_h = np.ascontiguousarray(
        fc2_w.T.reshape(FC, P, E).transpose(1, 0, 2)).astype(f32).astype(BF)
    misc_h = np.concatenate([
        (-r_eff).reshape(FC, P).T, b1_eff.reshape(FC, P).T],
        axis=1).astype(f32)
    misc_h = np.ascontiguousarray(misc_h)
    rows_h = np.ascontiguousarray(
        np.concatenate([bcomb, ncsum]).reshape(1, 2 * E).astype(f32))
    b2_h = np.ascontiguousarray(fc2_b.reshape(1, E)).astype(BF)

    def pack_te(a):  # [512 tok, 512 e] -> [128, 4, 512] partition-major
        return np.ascontiguousarray(a.reshape(OWN, P, E).transpose(1, 0, 2))

    in_maps = []
    for c in range(NCORES):
        b, half = divmod(c, 2)
        xown = x[b, half * HS:(half + 1) * HS]
        xoth = x[b, (1 - half) * HS:(2 - half) * HS]
        in_maps.append({
            "xob": pack_te(xown).astype(BF),
            "xhb": pack_te(xoth).astype(BF),
            "xot": np.ascontiguousarray(
                xown.T.reshape(EC, P, HS).transpose(1, 0, 2)).astype(BF),
            "xo32": pack_te(xown),
            "cw": cw_h, "w1": w1_h, "w2": w2_h,
            "misc": misc_h, "rows32": rows_h, "b2r": b2_h,
        })

    nc = _program()
    trace = os.environ.get("BASS_KERNEL_TRACE") == "1"
    res = run_bass_kernel_spmd(nc, in_maps, list(range(NCORES)), trace=trace)
    LAST_RESULT = res

    full = np.empty((B, S, E), f32)
    for c in range(NCORES):
        b, half = divmod(c, 2)
        o = res.results[c]["out"]                  # [128, 4, 512]
        full[b, half * HS:(half + 1) * HS, :] = (
            o.transpose(1, 0, 2).reshape(HS, E))
    return full
